# revision 1
# baseline (speedup 1.0000x reference)
"""EqualizedModConv2D (StyleGAN2 modulated conv) on 8 TRN2 NeuronCores.

Math rewrite (exact algebra, no approximation beyond matmul dtype):
    mod[n,i]  = style[n] @ (fc_weight * fc_scale).T[.,i] + bias[i] + 1
    out[n]    = demod_eff[n,:] * conv2d(mod[n,:] * x[n], weight)      (pad=1)
    demod_eff[n,o] = 1 / sqrt( sum_i mod[n,i]^2 * wsq[o,i] + eps/w_scale^2 )
    wsq[o,i]  = sum_{kh,kw} weight[o,i,kh,kw]^2
which equals the reference's per-sample-modulated-weight grouped conv with
w_scale and demodulation folded into input/output channel scalings.

Sharding: data-parallel over batch N=16 -> 2 samples per core; weights
replicated. Conv = 9 shifted fp32r matmuls over a zero-padded SBUF image,
accumulated in PSUM (4 ic-blocks x 9 taps = 36 matmuls per PSUM bank).

NOTE on structure: every tensor feeding a matmul (lhsT, rhs, and the PSUM
slot's previous reader) is produced on ScalarE (ACT). The self-loading
fp32/fp32r Matmult (S3_LW) only supports ONE sync-wait command in walrus
codegen; funneling all matmul dependencies through a single engine's
semaphore keeps every matmul at <=1 wait.
"""

import numpy as np

import concourse.bass as bass
import concourse.bacc as bacc
import concourse.tile as tile
from concourse import mybir
from concourse.bass_utils import run_bass_kernel_spmd

F32 = mybir.dt.float32
F32R = mybir.dt.float32r
BF16 = mybir.dt.bfloat16
AF = mybir.ActivationFunctionType
CONV_DT = F32R  # conv matmul operand dtype: F32R (precise) or BF16 (fast)

N_FULL, IC, OC, H, W = 16, 512, 512, 32, 32
DLAT, KS = 512, 3
NCORES = 8
NPC = N_FULL // NCORES          # samples per core
HP, WP = H + 2, W + 2           # padded image
FC_SCALE = 1.0 / float(np.sqrt(DLAT))
EPS_EFF = 1e-8 * (IC * KS * KS)  # eps / w_scale^2
NIB = IC // 128
NOB = OC // 128
NDB = DLAT // 128
HALF = 16                       # output rows per conv chain (N=16*32=512 fp32)

_NC = None


def _build(loop_iters=None):
    nc = bacc.Bacc()
    x_d = nc.declare_dram_parameter("x", [NPC, IC, H, W], F32, False)
    wt_d = nc.declare_dram_parameter("wt", [KS * KS, IC, OC], F32R, False)
    # pk packs [fcwT (512c) | styleT (NPC c) | bias (1c)] along the free dim
    pk_d = nc.declare_dram_parameter("pk", [DLAT, IC + NPC + 1], F32, False)
    out_d = nc.declare_dram_parameter("out", [NPC, OC, H, W], F32, True)

    import contextlib
    with tile.TileContext(nc) as tc:
        with (tc.For_i(0, loop_iters, 1,
                       hint_engines=(mybir.EngineType.PE,
                                     mybir.EngineType.Activation,
                                     mybir.EngineType.DVE,
                                     mybir.EngineType.SP))
              if loop_iters else contextlib.nullcontext()):
         with (
            tc.tile_pool(name="const", bufs=1) as cpool,
            tc.tile_pool(name="stg", bufs=2) as stg_pool,
            tc.tile_pool(name="xraw", bufs=3) as xraw_pool,
            tc.tile_pool(name="xpad", bufs=NPC * NIB) as xpad_pool,
            tc.tile_pool(name="wtp", bufs=8) as wt_pool,
            tc.tile_pool(name="tmp", bufs=2) as tmp_pool,
            tc.tile_pool(name="wsq", bufs=8) as wsq_pool,
            tc.tile_pool(name="outsb", bufs=4) as out_pool,
            tc.tile_pool(name="small", bufs=2) as small_pool,
            tc.tile_pool(name="cpsum", bufs=7, space="PSUM") as cpsum_pool,
            tc.tile_pool(name="spsum", bufs=1, space="PSUM") as spsum_pool,
        ):
            # ---------------- constants (pk tiles used directly by matmuls)
            fcw_sb, st_sb, b1_sb = [], [], []
            for d in range(NDB):
                ps = cpool.tile([128, IC + NPC + 1], F32, tag=f"pk{d}",
                                name=f"pk{d}")
                nc.sync.dma_start(out=ps[:], in_=pk_d[d * 128:(d + 1) * 128, :])
                fcw_sb.append(ps)
                st_sb.append(ps[:, IC:IC + NPC])
                t1 = cpool.tile([128, 1], F32, tag=f"b1{d}", name=f"b1{d}")
                nc.vector.tensor_scalar_add(t1[:], ps[:, IC + NPC:IC + NPC + 1], 1.0)
                b1_sb.append(t1)

            eps_sb = cpool.tile([128, 1], F32, tag="eps", name="eps")
            nc.vector.memset(eps_sb[:], float(EPS_EFF))
            zpad = cpool.tile([128, WP], F32, tag="zpad", name="zpad")
            nc.vector.memset(zpad[:], 0.0)

            # ---------------- mod / mod^2  (i on partitions, n free) --------
            mod_sb, mod2_sb = [], []
            for i in range(NIB):
                mp = spsum_pool.tile([128, NPC], F32, tag="sp", name=f"mp{i}")
                for d in range(NDB):
                    nc.tensor.matmul(
                        mp[:],
                        fcw_sb[d][:, i * 128:(i + 1) * 128],
                        st_sb[d],
                        start=(d == 0),
                        stop=(d == NDB - 1),
                    )
                m = cpool.tile([128, NPC], F32, tag=f"mod{i}", name=f"mod{i}")
                nc.scalar.activation(m[:], mp[:], AF.Identity,
                                     bias=b1_sb[i][:, 0:1], scale=FC_SCALE)
                m2 = cpool.tile([128, NPC], F32, tag=f"mod2{i}", name=f"mod2{i}")
                nc.scalar.square(m2[:], m[:])
                mod_sb.append(m)
                mod2_sb.append(m2)

            # ---------------- x: load, zero-pad, modulate (all writes on ACT)
            xpad = [[None] * NIB for _ in range(NPC)]
            for n in range(NPC):
                for i in range(NIB):
                    xr = xraw_pool.tile([128, H, W], F32, tag="xr", name=f"xr{n}_{i}")
                    nc.sync.dma_start(out=xr[:], in_=x_d[n, i * 128:(i + 1) * 128, :, :])
                    xp = xpad_pool.tile([128, HP, WP], CONV_DT, tag="xp", name=f"xp{n}_{i}")
                    nc.scalar.copy(xp[:, 0, :], zpad[:])
                    nc.scalar.copy(xp[:, HP - 1, :], zpad[:])
                    nc.scalar.copy(xp[:, 1:H + 1, 0:1], zpad[:, 0:H].rearrange("p (a b) -> p a b", b=1))
                    nc.scalar.copy(xp[:, 1:H + 1, WP - 1:WP], zpad[:, 0:H].rearrange("p (a b) -> p a b", b=1))
                    nc.scalar.mul(xp[:, 1:H + 1, 1:W + 1], xr[:], mod_sb[i][:, n:n + 1])
                    xpad[n][i] = xp

            # ---------------- per-oc-block: wsq, demod, conv ----------------
            for o in range(NOB):
                wts = []
                for i in range(NIB):
                    wt_t = wt_pool.tile([128, KS * KS, 128], CONV_DT, tag="wt",
                                        name=f"wt_o{o}i{i}")
                    nc.sync.dma_start(
                        out=wt_t[:],
                        in_=wt_d[:, i * 128:(i + 1) * 128,
                                 o * 128:(o + 1) * 128].transpose([1, 0, 2]),
                    )
                    wts.append(wt_t)

                # wsqT[i, o] = sum_k wt[k, i, o]^2
                wsqs = []
                for i in range(NIB):
                    tmp = tmp_pool.tile([128, 128, KS * KS], F32, tag="tmp",
                                        name=f"tmp_o{o}i{i}")
                    nc.scalar.square(tmp[:].transpose([0, 2, 1]), wts[i][:].bitcast(F32))
                    wq_s = stg_pool.tile([128, 128], F32, tag="wsq_stg",
                                         name=f"wsq_stg_o{o}i{i}")
                    nc.vector.tensor_reduce(wq_s[:], tmp[:],
                                            axis=mybir.AxisListType.X,
                                            op=mybir.AluOpType.add)
                    wq = wsq_pool.tile([128, 128], F32, tag="wsq",
                                       name=f"wsq_o{o}i{i}")
                    nc.scalar.copy(wq[:], wq_s[:])
                    wsqs.append(wq)

                dp = spsum_pool.tile([128, NPC], F32, tag="sp", name=f"dp{o}")
                for i in range(NIB):
                    nc.tensor.matmul(dp[:], wsqs[i][:], mod2_sb[i][:],
                                     start=(i == 0), stop=(i == NIB - 1))
                sq = small_pool.tile([128, NPC], F32, tag="sq", name=f"sq{o}")
                nc.scalar.activation(sq[:], dp[:], AF.Sqrt,
                                     bias=eps_sb[:, 0:1], scale=1.0)
                dem = small_pool.tile([128, NPC], F32, tag="dem", name=f"dem{o}")
                nc.vector.reciprocal(dem[:], sq[:])

                # conv: 4 chains (sample x image-half), 36 matmuls each
                chains = [(n, h) for n in range(NPC) for h in range(2)]
                psums = [
                    cpsum_pool.tile([128, HALF, W], F32, tag="cps",
                                    name=f"cps_o{o}c{ci}")
                    for ci in range(len(chains))
                ]
                for i in range(NIB):
                    for k in range(KS * KS):
                        kh, kw = divmod(k, KS)
                        lw = wts[i][:, k, :]
                        first = (i == 0 and k == 0)
                        last = (i == NIB - 1 and k == KS * KS - 1)
                        for ci, (n, h) in enumerate(chains):
                            y0 = h * HALF
                            rhs = xpad[n][i][:, kh + y0:kh + y0 + HALF,
                                             kw:kw + W]
                            nc.tensor.matmul(psums[ci][:], lw, rhs,
                                             start=first, stop=last)
                for ci, (n, h) in enumerate(chains):
                    ob = out_pool.tile([128, HALF, W], F32, tag="ob",
                                       name=f"ob_o{o}c{ci}")
                    nc.scalar.mul(ob[:], psums[ci][:], dem[:, n:n + 1])
                    nc.sync.dma_start(
                        out=out_d[n, o * 128:(o + 1) * 128,
                                  h * HALF:(h + 1) * HALF, :],
                        in_=ob[:],
                    )
    nc.finalize()
    return nc


def _get_nc():
    global _NC
    if _NC is None:
        _NC = _build()
    return _NC


def _make_in_maps(x, style, weight, fc_weight, bias):
    x = np.ascontiguousarray(np.asarray(x, np.float32))
    wt = np.ascontiguousarray(
        np.asarray(weight, np.float32).transpose(2, 3, 1, 0).reshape(KS * KS, IC, OC))
    styleT = np.asarray(style, np.float32).T
    fcwT = np.asarray(fc_weight, np.float32).T
    biasr = np.asarray(bias, np.float32).reshape(IC, 1)
    in_maps = []
    for c in range(NCORES):
        pk = np.ascontiguousarray(np.concatenate(
            [fcwT, styleT[:, c * NPC:(c + 1) * NPC], biasr], axis=1))
        in_maps.append({
            "x": np.ascontiguousarray(x[c * NPC:(c + 1) * NPC]),
            "wt": wt,
            "pk": pk,
        })
    return in_maps


def _run(in_maps, trace=False):
    last = None
    for _ in range(3):
        try:
            return run_bass_kernel_spmd(_get_nc(), in_maps, list(range(NCORES)),
                                        trace=trace)
        except Exception as e:  # transient NRT/device errors: retry
            last = e
    raise last


def kernel(x, style, weight, fc_weight, bias):
    br = _run(_make_in_maps(x, style, weight, fc_weight, bias))
    out = np.concatenate([br.results[c]["out"] for c in range(NCORES)], axis=0)
    return out


def _make_runner(nc, in_maps):
    import jax
    import numpy as np
    from jax.sharding import Mesh, PartitionSpec
    from jax.experimental.shard_map import shard_map
    from concourse import mybir as _mb
    from concourse.bass2jax import (_bass_exec_p, install_neuronx_cc_hook,
                                    partition_id_tensor)
    install_neuronx_cc_hook()
    n_cores = len(in_maps)
    partition_name = nc.partition_id_tensor.name if nc.partition_id_tensor else None
    in_names, out_names, out_avals, zero_outs = [], [], [], []
    for alloc in nc.m.functions[0].allocations:
        if not isinstance(alloc, _mb.MemoryLocationSet):
            continue
        name = alloc.memorylocations[0].name
        if alloc.kind == "ExternalInput":
            if name != partition_name:
                in_names.append(name)
        elif alloc.kind == "ExternalOutput":
            shape = tuple(alloc.tensor_shape)
            dtype = _mb.dt.np(alloc.dtype)
            out_avals.append(jax.core.ShapedArray(shape, dtype))
            out_names.append(name)
            zero_outs.append(np.zeros(shape, dtype))
    n_params = len(in_names)
    all_in_names = list(in_names) + list(out_names)
    if partition_name is not None:
        all_in_names.append(partition_name)

    def _body(*args):
        operands = list(args)
        if partition_name is not None:
            operands.append(partition_id_tensor())
        outs = _bass_exec_p.bind(
            *operands,
            out_avals=tuple(out_avals),
            in_names=tuple(all_in_names),
            out_names=tuple(out_names),
            lowering_input_output_aliases=(),
            sim_require_finite=True,
            sim_require_nnan=True,
            nc=nc,
        )
        return tuple(outs)

    devices = jax.devices()[:n_cores]
    mesh = Mesh(np.asarray(devices), ("core",))
    in_specs = (PartitionSpec("core"),) * (n_params + len(out_names))
    out_specs = (PartitionSpec("core"),) * len(out_names)
    fn = jax.jit(shard_map(_body, mesh=mesh, in_specs=in_specs,
                           out_specs=out_specs, check_rep=False))
    concat = []
    for nm in in_names:
        per = [np.asarray(in_maps[c][nm]) for c in range(n_cores)]
        concat.append(np.concatenate(per, axis=0))
    concat += [np.zeros((n_cores * z.shape[0], *z.shape[1:]), z.dtype)
               for z in zero_outs]
    args = [jax.device_put(a) for a in concat]
    return fn, args


def _time_runner(fn, args, iters, reps):
    import time
    import jax
    o = fn(*args)
    jax.block_until_ready(o)  # compile + warm
    best = float("inf")
    for _ in range(reps):
        t0 = time.perf_counter()
        for _ in range(iters):
            o = fn(*args)
            jax.block_until_ready(o)
        best = min(best, (time.perf_counter() - t0) / iters)
    return best


_NC_LOOP = None
_LOOP_R = 128


def measure_hw(inputs, iters=6, reps=3):
    """Differential HW timing: wall(body x R in a hardware loop) minus
    wall(body x 1), divided by R-1. Removes the ~120 ms axon dispatch
    overhead. Returns (per_iter_ns, (wall_plain_ns, wall_loop_ns))."""
    global _NC_LOOP
    in_maps = _make_in_maps(**inputs)
    fn1, args1 = _make_runner(_get_nc(), in_maps)
    if _NC_LOOP is None:
        _NC_LOOP = _build(loop_iters=_LOOP_R)
    fnR, argsR = _make_runner(_NC_LOOP, in_maps)
    w1 = _time_runner(fn1, args1, iters, reps) * 1e9
    wR = _time_runner(fnR, argsR, iters, reps) * 1e9
    per_iter = (wR - w1) / (_LOOP_R - 1)
    return per_iter, (w1, wR)


def predict_ns():
    """Cost-model (TimelineSim) predicted single-core kernel duration in ns."""
    from concourse.timeline_sim import TimelineSim
    ts = TimelineSim(_get_nc(), no_exec=True)
    return ts.simulate()


def run_profiled(inputs):
    """Dev helper: run with NTFF tracing; returns BassKernelResults."""
    return _run(_make_in_maps(**inputs), trace=True)

